# revision 25
# baseline (speedup 1.0000x reference)
"""Chamfer distance kernel for Trainium2 (Bass/Tile), 8 NeuronCores.

Full inputs: xyz1 [8, 4096, 3] f32, xyz2 [8, 4096, 3] f32.
Output: scalar f32 = mean(min_m d2[b,n,m]) + mean(min_n d2[b,n,m]).

Sharding: data-parallel over batch B=8, one batch element per core.
Each core computes partial sums [sum(dist1), sum(dist2)]; host combines
(and negates: the device works on -d2 throughout, see below).

Per-core algorithm: the NEGATED d2 matrix is produced directly by a K=9
f32r matmul (f32r = fp32 bits, 4x faster PE streaming) over augmented
operands that are host-side layouts of the input points:
  aug1 rows = [x1_d (3), x1_d^2 (3), 1 (3)]      (lhsT, [9, 4096])
  aug2 rows = [2*x2_d (3), -1 (3), -x2_d^2 (3)]  (rhs,  [9, 4096])
  psum[n, m] = -(||x1_n||^2 - 2 x1_n.x2_m + ||x2_m||^2) = -d2
Everything downstream is a MAX fold (min of d2 == max of -d2), which is
what the engines support best. Per [128 x 4096] row-block (n-chunk):
  - PE writes two [128 x 2048] PSUM tiles (4 matmuls each, one per bank);
  - ACT downcasts both tiles into a bf16 staging buffer (SBUF).
DVE (the bottleneck engine) then processes staging in GROUPS of up to 4
row-blocks so each instruction is maximally wide (per-op overhead is
~250ns): a 4-level 2x-mode bf16 tensor_tensor fold pyramid shrinks the
1x-mode rowmax reduce to 1/16 width (dist1), and a pairwise-tree + one
running max per group accumulates the dist2 plane R[128, 4096].
Progressive group sizes [2,2,4...] shorten the pipeline-fill ramp.
Epilogue: GPSIMD partition_all_reduce(max) folds R across partitions in
quarter-planes (each starting as soon as the last group's quarter-fold
lands, overlapping the DVE row-sums); dist1's partition-sum uses a tiny
PE ones-matmul.
Cost-model timeline: ~173 us/core total; DVE ~156 us busy, ACT ~136 us,
PE ~70 us. Reduce ops have no DVE perf modes (always 1 elem/lane/cycle),
which is why 2x-mode tensor_tensor prefolds carry most of the fold work.
"""

import numpy as np
from contextlib import ExitStack

import concourse.bass as bass
import concourse.bacc as bacc
import concourse.bass_isa as bass_isa
import concourse.mybir as mybir
from concourse.tile import TileContext
from concourse.bass_utils import run_bass_kernel_spmd

B, N, M, D = 8, 4096, 4096, 3
P = 128            # partitions (n-chunk size)
NI = N // P        # 32 n-chunks
FREE = 2048        # columns per DVE op (4 PSUM banks)
NJ = M // FREE     # 2 column groups
MM = 512           # matmul free dim (1 PSUM bank)
FDT = mybir.dt.float32
FRT = mybir.dt.float32r   # same bits as f32; PE streams 1 row/cycle (vs 4)
BDT = mybir.dt.bfloat16
AX = mybir.AxisListType
MAX = mybir.AluOpType.max
ADD = mybir.AluOpType.add

_CACHE = {}


def _build():
    nc = bacc.Bacc(None, target_bir_lowering=False)
    a1 = nc.dram_tensor("aug1", [9, N], FRT, kind="ExternalInput")
    a2 = nc.dram_tensor("aug2", [9, M], FRT, kind="ExternalInput")
    part = nc.dram_tensor("partial", [1, 2], FDT, kind="ExternalOutput")

    with ExitStack() as ctx:
        tc = ctx.enter_context(TileContext(nc))
        sb = ctx.enter_context(tc.tile_pool(name="sb", bufs=1))
        stg = ctx.enter_context(tc.tile_pool(name="stg", bufs=2))
        stf = ctx.enter_context(tc.tile_pool(name="stf", bufs=1))
        pp = ctx.enter_context(tc.tile_pool(name="pp", bufs=2, space="PSUM"))

        aug1 = sb.tile([9, N], FRT)
        aug2 = sb.tile([9, M], FRT)
        nc.sync.dma_start(out=aug1[:, :], in_=a1[:, :])
        nc.sync.dma_start(out=aug2[:, :], in_=a2[:, :])

        R = sb.tile([P, M], BDT)           # running max over n-chunks, per m
        D1 = sb.tile([P, NI], FDT)         # full-row max per n-chunk

        # ---- main loop ----
        # Per tile: PE matmul (f32r) -> PSUM; ACT downcasts PSUM -> bf16
        # staging. DVE (the bottleneck) runs everything in bf16, batched
        # over groups of G n-chunks so each instruction is as wide as
        # possible (DVE op overhead ~250ns): a 4-level 2x-mode fold
        # pyramid shrinks the 1x-mode rowmax-reduce to 1/16 width, and a
        # pairwise tree + one running max folds the dist2 plane.
        # Progressive group sizes: DVE starts working after one chunk's
        # copies instead of idling through a full 4-chunk group.
        GROUPS = [2, 2] + [4] * 7
        assert sum(GROUPS) == NI
        # fold-pyramid depth per group size (deeper batching amortizes the
        # ~250ns DVE per-op overhead)
        LEVELS = {1: 2, 2: 3, 4: 4}

        i0 = 0
        for gidx, G in enumerate(GROUPS):
            st = stg.tile([P, G, M], BDT, tag="st")
            for c in range(G):
                i = i0 + c
                for jh in range(NJ):
                    pt = pp.tile([P, FREE], FDT, tag="pt")
                    for k in range(FREE // MM):
                        nc.tensor.matmul(
                            pt[:, k * MM:(k + 1) * MM],
                            aug1[:, i * P:(i + 1) * P],
                            aug2[:, jh * FREE + k * MM: jh * FREE + (k + 1) * MM],
                            start=True, stop=True,
                        )
                    nc.scalar.copy(
                        st[:, c, jh * FREE:(jh + 1) * FREE], pt[:, :],
                    )
            # dist1 batched fold pyramid (per-chunk row maxes kept separate)
            prev, w = st, M
            for lvl in range(LEVELS[G]):
                nxt = stf.tile([P, G, w // 2], BDT, tag=f"f{lvl}")
                nc.vector.tensor_tensor(
                    out=nxt[:, :, :], in0=prev[:, :, 0:w // 2],
                    in1=prev[:, :, w // 2:w], op=MAX,
                )
                prev, w = nxt, w // 2
            nc.vector.tensor_reduce(
                out=D1[:, i0:i0 + G], in_=prev[:, :, :], axis=AX.X, op=MAX,
            )
            # dist2 pairwise tree within the group, then fold into R
            cur, width = st, G
            while width > 1:
                cv = cur[:, :, :].rearrange("p (a b) m -> p a b m", a=width // 2)
                nxt = stf.tile([P, width // 2, M], BDT, tag=f"t{width}")
                nc.vector.tensor_tensor(
                    out=nxt[:, :, :], in0=cv[:, :, 0, :], in1=cv[:, :, 1, :], op=MAX,
                )
                cur, width = nxt, width // 2
            t2 = cur[:, 0, :]
            if gidx == 0:
                nc.vector.tensor_copy(out=R[:, :], in_=t2)
            elif gidx == len(GROUPS) - 1:
                # last group: fold in quarter-planes so the epilogue's
                # GPSIMD partition folds can start before the full plane
                # is finished
                q = M // 4
                for qq in range(4):
                    nc.vector.tensor_tensor(
                        out=R[:, qq * q:(qq + 1) * q], in0=t2[:, qq * q:(qq + 1) * q],
                        in1=R[:, qq * q:(qq + 1) * q], op=MAX,
                    )
            else:
                nc.vector.tensor_tensor(out=R[:, :], in0=t2, in1=R[:, :], op=MAX)
            i0 += G

        # ---- dist1 epilogue: sum over i, partition-sum via PE ----
        s1 = sb.tile([P, 1], FDT)
        nc.vector.tensor_reduce(out=s1[:, :], in_=D1[:, :], axis=AX.X, op=ADD)
        ones_col = sb.tile([P, 1], FDT)
        nc.vector.memset(ones_col[:, :], 1.0)
        out_t = sb.tile([1, 2], FDT)

        p1 = pp.tile([1, 1], FDT, tag="pt")
        nc.tensor.matmul(p1[:, :], s1[:, :], ones_col[:, :], start=True, stop=True)
        nc.vector.tensor_copy(out=out_t[0:1, 0:1], in_=p1[0:1, 0:1])

        # ---- dist2 epilogue: GPSIMD partition fold, then row-sum ----
        # quarter-planes: each Pool fold starts as soon as its quarter of
        # R is final, and each DVE row-sum overlaps the next Pool fold
        Rr = sb.tile([P, M], BDT)
        q = M // 4
        s2h = sb.tile([1, 4], FDT)
        for qq in range(4):
            nc.gpsimd.partition_all_reduce(
                Rr[:, qq * q:(qq + 1) * q], R[:, qq * q:(qq + 1) * q],
                P, bass_isa.ReduceOp.max,
            )
        for qq in range(4):
            nc.vector.tensor_reduce(
                out=s2h[0:1, qq:qq + 1], in_=Rr[0:1, qq * q:(qq + 1) * q],
                axis=AX.X, op=ADD,
            )
        nc.vector.tensor_reduce(
            out=out_t[0:1, 1:2], in_=s2h[0:1, :], axis=AX.X, op=ADD,
        )

        nc.sync.dma_start(out=part[:, :], in_=out_t[0:1, :])

    nc.compile()
    return nc


def _get_nc():
    if "nc" not in _CACHE:
        _CACHE["nc"] = _build()
    return _CACHE["nc"]


def _augment(xyz1_b, xyz2_b):
    """Host-side layout of one batch element into the augmented operands."""
    a1 = np.empty((9, N), dtype=np.float32)
    t1 = xyz1_b.T.astype(np.float32)           # [3, N]
    a1[0:3] = t1
    a1[3:6] = t1 * t1
    a1[6:9] = 1.0
    a2 = np.empty((9, M), dtype=np.float32)
    t2 = xyz2_b.T.astype(np.float32)           # [3, M]
    # negated so the PE emits -d2: all on-chip folds become MAX
    # (GPSIMD only implements tensor_max, not min)
    a2[0:3] = 2.0 * t2
    a2[3:6] = -1.0
    a2[6:9] = -(t2 * t2)
    return a1, a2


def run_cores(xyz1, xyz2, **kw):
    """Run the per-core kernel on all 8 cores; returns BassKernelResults."""
    xyz1 = np.asarray(xyz1, dtype=np.float32)
    xyz2 = np.asarray(xyz2, dtype=np.float32)
    assert xyz1.shape == (B, N, D) and xyz2.shape == (B, M, D)
    in_maps = []
    for b in range(B):
        a1, a2 = _augment(xyz1[b], xyz2[b])
        in_maps.append({"aug1": a1, "aug2": a2})
    return run_bass_kernel_spmd(_get_nc(), in_maps, list(range(B)), **kw)


def _combine(results):
    parts = np.stack([r["partial"][0] for r in results])  # [8, 2]
    s1 = float(parts[:, 0].astype(np.float64).sum())
    s2 = float(parts[:, 1].astype(np.float64).sum())
    return np.asarray(-(s1 / (B * N) + s2 / (B * M)), dtype=np.float32)


def kernel(xyz1, xyz2):
    res = run_cores(xyz1, xyz2)
    return _combine(res.results)

